# revision 1
# baseline (speedup 1.0000x reference)
"""Trainium2 Bass kernel for DiffusionOperator (polynomial graph diffusion).

result = sum_k coeffs[k] * T^k x,  T = D^-1/2 A D^-1/2 (deg by edge col/source),
coeffs = softmax(MLP(graph stats)).

Strategy (8 NeuronCores, SPMD):
  * Nodes partitioned into 8 contiguous slices of R=12500 (dest/row side).
  * Reformulation: s_0 = dis*x; a_k = A @ s_{k-1} (plain 0/1 adjacency,
    summed by destination); s_k = dis^2 * a_k; result = c0*x + (sum_k c_k s_k)/dis.
    This removes all per-edge weights: the per-edge work is a pure gather +
    one-hot matmul segment-sum; dis scaling is per-node (cheap).
  * Per step: every core gathers s_{k-1}[col] for its edges from a replicated
    full table in its HBM (dma_gather, 256B rows), segment-sums by dest via
    PE matmuls with on-chip-built one-hot matrices, writes its new 12500-row
    slice, and an AllGather rebuilds the replicated table for the next step.
  * Edge index preprocessing (sort/pad/layout) is host-side numpy; all float
    math on x flows through the device kernel.

Self-contained: hardcodes full-problem shapes; builds/compiles on first call.
"""

import math
import os
import sys
from dataclasses import dataclass

import numpy as np

for _p in ("/opt/trn_rl_repo",):
    if _p not in sys.path:
        sys.path.insert(0, _p)

import concourse.bacc as bacc
import concourse.bass as bass
import concourse.bass_isa as bass_isa
import concourse.mybir as mybir
import concourse.tile as tile
from concourse import tile_sem_assignment as _tsa
from concourse.masks import make_identity
from concourse.tile_scheduler import DMAInst as _DMAInst


def _install_queue_aware_dmasw():
    """Map Pool SWDGE DMAs to DMASW lanes by queue_num (lane = q + 4*(i%2))
    so multi-queue dma_gather passes the per-queue semaphore-lock check."""
    if getattr(_tsa.TileClockTick, "_qaware", False):
        return
    orig = _tsa.TileClockTick._assign_tick

    def patched(self, inst):
        if (
            isinstance(inst, _DMAInst)
            and inst.engine == mybir.EngineType.Pool
            and not isinstance(inst, bass_isa.UserSyncedRemoteDMADescs)
        ):
            qn = int(getattr(inst, "queue_num", 0) or 0)
            ctr = self.__dict__.setdefault("_qctr", {})
            c = ctr.get(qn, 0)
            ctr[qn] = c + 1
            self.next_sw_dma_idx = qn + 4 * (c % 2)
        return orig(self, inst)

    _tsa.TileClockTick._assign_tick = patched
    _tsa.TileClockTick._qaware = True


_install_queue_aware_dmasw()

F32 = mybir.dt.float32
I16 = mybir.dt.int16
AF = mybir.ActivationFunctionType
ALU = mybir.AluOpType
P = 128


@dataclass(frozen=True)
class Cfg:
    N: int          # nodes
    E: int          # edges
    C: int          # channels (64)
    H: int          # mlp hidden (32)
    K: int          # poly degree (5)
    ncores: int     # 8
    nq: int         # source quadrants for int16 gather indices
    piece_tok: int  # tokens per dma_gather piece (HW cap: 1024 idxs/gather)
    c_need: int     # chunks (x128 tokens) per (block, quadrant)

    @property
    def R(self):  # rows per core
        return self.N // self.ncores

    @property
    def NB(self):  # dest blocks per core
        return math.ceil(self.R / P)

    @property
    def tail(self):  # real rows in last block
        return self.R - (self.NB - 1) * P

    @property
    def QS(self):  # rows per source quadrant
        return self.N // self.nq

    @property
    def cap(self):  # tokens per (block, quadrant)
        return self.c_need * P

    @property
    def LQ(self):  # tokens per (core, quadrant) stream
        return self.NB * self.cap

    @property
    def LQP(self):  # padded stream length per quadrant (x piece_tok)
        return math.ceil(self.LQ / self.piece_tok) * self.piece_tok

    @property
    def n_pieces(self):  # gather pieces per quadrant
        return self.LQP // self.piece_tok

    @property
    def nchunk(self):  # total chunk columns in dcols
        return self.nq * self.NB * self.c_need


FULL = dict(N=100000, E=1600000, C=64, H=32, K=5, ncores=8, nq=4, piece_tok=1024)


def _preprocess(x, edge_index, cfg_kw):
    """Host-side index preprocessing -> per-core input maps + Cfg."""
    N, E, ncores, nq = cfg_kw["N"], cfg_kw["E"], cfg_kw["ncores"], cfg_kw["nq"]
    R = N // ncores
    NB = math.ceil(R / P)
    QS = N // nq
    row = np.asarray(edge_index[0], dtype=np.int64)
    col = np.asarray(edge_index[1], dtype=np.int64)
    deg = np.bincount(col, minlength=N).astype(np.float32)

    m = row // R
    b = (row % R) // P
    dl = (row % R) % P
    q = col // QS
    lidx = (col - q * QS).astype(np.int16)

    ngroups = ncores * nq * NB
    key = (m * nq + q) * NB + b
    order = np.argsort(key, kind="stable")
    counts = np.bincount(key, minlength=ngroups)
    c_need = int(math.ceil(counts.max() / P))
    cap = c_need * P

    starts = np.zeros(ngroups, dtype=np.int64)
    np.cumsum(counts[:-1], out=starts[1:])
    rank = np.arange(E, dtype=np.int64) - np.repeat(starts, counts)
    pos = np.repeat(np.arange(ngroups, dtype=np.int64) * cap, counts) + rank

    idx_all = np.zeros(ngroups * cap, dtype=np.int16)
    dst_all = np.full(ngroups * cap, 255.0, dtype=np.float32)
    idx_all[pos] = lidx[order]
    dst_all[pos] = dl[order].astype(np.float32)

    cfg = Cfg(c_need=c_need, **cfg_kw)
    LQ, LQP = cfg.LQ, cfg.LQP

    dst_all = (
        dst_all.reshape(ncores, nq, NB, c_need, P)
        .transpose(0, 2, 1, 3, 4)
        .reshape(ncores, NB * nq * c_need, P)
    )
    idx_all = idx_all.reshape(ncores, nq, LQ)
    if LQP > LQ:
        idx_all = np.concatenate(
            [idx_all, np.zeros((ncores, nq, LQP - LQ), np.int16)], axis=2
        )

    in_maps = []
    w = {}  # filled by caller with mlp weights
    for mm in range(ncores):
        # wrapped int16 indices: token i of stream q -> [i%16, i//16]; replicate
        # the 16-partition pattern across all 128 partitions (8 gpsimd cores).
        wr = np.concatenate(
            [
                np.tile(idx_all[mm, qq].reshape(LQP // 16, 16).T, (8, 1))
                for qq in range(nq)
            ],
            axis=1,
        )  # [128, nq*LQ//16]
        degp = np.ones(NB * P, dtype=np.float32)
        degp[:R] = deg[mm * R : (mm + 1) * R]
        in_maps.append(
            {
                "xs": np.ascontiguousarray(x[mm * R : (mm + 1) * R]).astype(np.float32),
                "degp": np.ascontiguousarray(degp.reshape(NB, P).T),
                "dcols": np.ascontiguousarray(dst_all[mm].T),
                "idx": np.ascontiguousarray(wr),
            }
        )
    return cfg, in_maps


def _build_program(cfg: Cfg):
    nc = bacc.Bacc("TRN2", num_swdge_queues=cfg.nq)
    C, NB, R, K = cfg.C, cfg.NB, cfg.R, cfg.K
    FB = NB * C  # free bytes per stage row (elements)
    nmain = (NB - 1) * P  # rows covered by full blocks

    xs_t = nc.declare_dram_parameter("xs", [R, C], F32, isOutput=False)
    degp_t = nc.declare_dram_parameter("degp", [P, NB], F32, isOutput=False)
    dcols_t = nc.declare_dram_parameter("dcols", [P, cfg.nchunk], F32, isOutput=False)
    idx_t = nc.declare_dram_parameter(
        "idx", [P, cfg.nq * (cfg.LQP // 16)], I16, isOutput=False
    )
    w1t_t = nc.declare_dram_parameter("w1t", [C + 4, cfg.H], F32, isOutput=False)
    b1c_t = nc.declare_dram_parameter("b1c", [cfg.H, 1], F32, isOutput=False)
    w2t_t = nc.declare_dram_parameter("w2t", [cfg.H, K + 1], F32, isOutput=False)
    b2r_t = nc.declare_dram_parameter("b2r", [1, K + 1], F32, isOutput=False)
    out_t = nc.declare_dram_parameter("out", [R, C], F32, isOutput=True)

    s_loc = [nc.dram_tensor(f"s_loc{k}", [R, C], F32) for k in range(K + 1)]
    table = [
        nc.dram_tensor(f"table{k}", [cfg.N, C], F32, addr_space="Shared")
        for k in range(K)
    ]
    stats_loc = nc.dram_tensor("stats_loc", [136], F32)
    stats_red = nc.dram_tensor("stats_red", [136], F32, addr_space="Shared")

    groups = [list(range(cfg.ncores))]

    def dram_pf(t):
        """[R, C] dram tensor viewed [P, NB-1, C] (p-major enum) + tail view."""
        main = t[0:nmain, :].rearrange("(b p) f -> p b f", p=P)
        tl = t[nmain:R, :]
        return main, tl

    with tile.TileContext(nc) as tc:
        with (
            tc.tile_pool(name="const", bufs=1) as cpool,
            tc.tile_pool(name="stage", bufs=3) as stpool,
            tc.tile_pool(name="gp", bufs=16) as gpool,
            tc.tile_pool(name="sp", bufs=3) as spool,
            tc.tile_pool(name="small", bufs=2) as smpool,
            tc.tile_pool(name="pmain", bufs=6, space="PSUM") as pmain,
            tc.tile_pool(name="psmall", bufs=2, space="PSUM") as psmall,
        ):
            # ---- constants ----
            iota_t = cpool.tile([P, cfg.nq * cfg.cap], F32)
            nc.gpsimd.iota(
                iota_t[:],
                [[0, cfg.nq * cfg.c_need], [1, P]],
                channel_multiplier=0,
                allow_small_or_imprecise_dtypes=True,
            )
            ident64 = cpool.tile([C, C], F32)
            make_identity(nc, ident64[:])
            ones_col = cpool.tile([P, 1], F32)
            nc.gpsimd.memset(ones_col[:], 1.0)
            ones_row = cpool.tile([1, P], F32)
            nc.gpsimd.memset(ones_row[:], 1.0)

            idxs = cpool.tile([P, cfg.nq * (cfg.LQP // 16)], I16)
            nc.sync.dma_start(out=idxs[:], in_=idx_t[:])
            dcols = cpool.tile([P, cfg.nchunk], F32)
            nc.sync.dma_start(out=dcols[:], in_=dcols_t[:])
            degp = cpool.tile([P, NB], F32)
            nc.sync.dma_start(out=degp[:], in_=degp_t[:])
            w1t = cpool.tile([C + 4, cfg.H], F32)
            nc.sync.dma_start(out=w1t[:], in_=w1t_t[:])
            b1c = cpool.tile([cfg.H, 1], F32)
            nc.sync.dma_start(out=b1c[:], in_=b1c_t[:])
            w2t = cpool.tile([cfg.H, K + 1], F32)
            nc.sync.dma_start(out=w2t[:], in_=w2t_t[:])
            b2r = cpool.tile([1, K + 1], F32)
            nc.sync.dma_start(out=b2r[:], in_=b2r_t[:])

            # dis = min(deg^-0.5, 1e6); dis2 = dis^2; rdis = 1/dis
            dis = cpool.tile([P, NB], F32)
            nc.scalar.activation(dis[:], degp[:], AF.Sqrt)
            nc.vector.tensor_scalar_max(dis[:], dis[:], 1.0e-6)
            nc.vector.reciprocal(dis[:], dis[:])
            dis2 = cpool.tile([P, NB], F32)
            nc.vector.tensor_tensor(dis2[:], dis[:], dis[:], op=ALU.mult)
            rdis = cpool.tile([P, NB], F32)
            nc.vector.reciprocal(rdis[:], dis[:])

            # ---- phase A: load x, stats partials, s0 ----
            x_t = stpool.tile([P, FB], F32, tag="stage")
            nc.gpsimd.memset(x_t[:], 0.0)
            xm, xtl = dram_pf(xs_t)
            nc.sync.dma_start(
                out=x_t[:, 0 : (NB - 1) * C].rearrange("p (b f) -> p b f", f=C),
                in_=xm,
            )
            nc.sync.dma_start(
                out=x_t[0 : cfg.tail, (NB - 1) * C : FB], in_=xtl
            )

            # per-channel sums over this core's rows: accumulate X_b^T @ ones
            csum_ps = psmall.tile([P, C], F32, tag="sm")
            for b in range(NB):
                nc.tensor.matmul(
                    csum_ps[0:C, 0:1],
                    lhsT=x_t[:, b * C : (b + 1) * C],
                    rhs=ones_col[:],
                    start=(b == 0),
                    stop=(b == NB - 1),
                )
            csum_sb = smpool.tile([C, 1], F32)
            nc.vector.tensor_copy(csum_sb[:], csum_ps[0:C, 0:1])

            # sum of squares via diag(X^T X): accumulate X_b^T @ X_b
            sq_ps = psmall.tile([P, C], F32, tag="sm")
            for b in range(NB):
                nc.tensor.matmul(
                    sq_ps[0:C, 0:C],
                    lhsT=x_t[:, b * C : (b + 1) * C],
                    rhs=x_t[:, b * C : (b + 1) * C],
                    start=(b == 0),
                    stop=(b == NB - 1),
                )
            sqd = smpool.tile([C, C], F32)
            nc.vector.tensor_tensor(sqd[:], sq_ps[0:C, 0:C], ident64[:], op=ALU.mult)
            sqch = smpool.tile([C, 1], F32)
            nc.vector.tensor_reduce(sqch[:], sqd[:], axis=mybir.AxisListType.X, op=ALU.add)

            zpad = smpool.tile([1, 8], F32, tag="zp")
            nc.gpsimd.memset(zpad[:], 0.0)
            nc.sync.dma_start(out=stats_loc[0:C], in_=csum_sb[:])
            nc.sync.dma_start(out=stats_loc[C : 2 * C], in_=sqch[:])
            nc.sync.dma_start(out=stats_loc[2 * C : 2 * C + 8], in_=zpad[:])
            nc.gpsimd.collective_compute(
                "AllReduce",
                ALU.add,
                replica_groups=groups,
                ins=[stats_loc[:]],
                outs=[stats_red[:]],
            )

            # s0 = dis * x  (blockwise per-partition scale on ACT)
            s_st = stpool.tile([P, FB], F32, tag="stage")
            for b in range(NB):
                nc.scalar.activation(
                    s_st[:, b * C : (b + 1) * C],
                    x_t[:, b * C : (b + 1) * C],
                    AF.Copy,
                    scale=dis[:, b : b + 1],
                )
            sm0, stl0 = dram_pf(s_loc[0])
            nc.sync.dma_start(
                out=sm0, in_=s_st[:, 0 : (NB - 1) * C].rearrange("p (b f) -> p b f", f=C)
            )
            nc.sync.dma_start(out=stl0, in_=s_st[0 : cfg.tail, (NB - 1) * C : FB])
            nc.gpsimd.collective_compute(
                "AllGather",
                ALU.bypass,
                replica_groups=groups,
                ins=[s_loc[0][:]],
                outs=[table[0][:]],
            )

            # ---- coeff MLP (runs concurrently with diffusion steps) ----
            red = smpool.tile([1, 136], F32)
            nc.sync.dma_start(out=red[:], in_=stats_red[:])
            cin = smpool.tile([P, 1], F32, tag="cin")
            nc.sync.dma_start(out=cin[0:C, 0:1], in_=red[0:1, 0:C])
            nc.vector.tensor_scalar_mul(cin[0:C, 0:1], cin[0:C, 0:1], 1.0 / cfg.N)
            M = float(cfg.N * cfg.C)
            mean = smpool.tile([1, 1], F32, tag="m1")
            nc.vector.tensor_reduce(
                mean[:], red[0:1, 0:C], axis=mybir.AxisListType.X, op=ALU.add
            )
            nc.scalar.mul(mean[:], mean[:], 1.0 / M)
            sqred = smpool.tile([1, 1], F32, tag="m2")
            nc.vector.tensor_reduce(
                sqred[:], red[0:1, C : 2 * C], axis=mybir.AxisListType.X, op=ALU.add
            )
            msq = smpool.tile([1, 1], F32, tag="m3")
            nc.vector.tensor_tensor(msq[:], mean[:], mean[:], op=ALU.mult)
            nc.scalar.mul(msq[:], msq[:], -M)
            nc.vector.tensor_tensor(msq[:], sqred[:], msq[:], op=ALU.add)
            nc.scalar.mul(msq[:], msq[:], 1.0 / (M - 1.0))
            nc.scalar.activation(msq[:], msq[:], AF.Sqrt)  # std
            srow = smpool.tile([1, 4], F32, tag="m4")
            nc.vector.tensor_copy(srow[0:1, 0:1], mean[:])
            nc.vector.tensor_copy(srow[0:1, 1:2], msq[:])
            nc.gpsimd.memset(srow[0:1, 2:3], float(cfg.N))
            nc.gpsimd.memset(srow[0:1, 3:4], float(cfg.E))
            nc.gpsimd.dma_start(out=cin[C : C + 4, 0:1], in_=srow[:])

            h_ps = psmall.tile([P, C], F32, tag="sm")
            nc.tensor.matmul(
                h_ps[0 : cfg.H, 0:1], lhsT=w1t[:], rhs=cin[0 : C + 4, 0:1],
                start=True, stop=True,
            )
            h_sb = smpool.tile([cfg.H, 1], F32, tag="h")
            nc.scalar.activation(h_sb[:], h_ps[0 : cfg.H, 0:1], AF.Relu, bias=b1c[:])
            c_ps = psmall.tile([P, C], F32, tag="sm")
            nc.tensor.matmul(
                c_ps[0:1, 0 : K + 1], lhsT=h_sb[:], rhs=w2t[:], start=True, stop=True
            )
            z = smpool.tile([1, K + 1], F32, tag="z")
            nc.vector.tensor_tensor(z[:], c_ps[0:1, 0 : K + 1], b2r[:], op=ALU.add)
            zmax = smpool.tile([1, 1], F32, tag="m5")
            nc.vector.tensor_reduce(zmax[:], z[:], axis=mybir.AxisListType.X, op=ALU.max)
            nc.vector.tensor_scalar(
                z[:], z[:], zmax[0:1, 0:1], None, op0=ALU.subtract
            )
            nc.scalar.activation(z[:], z[:], AF.Exp)
            zsum = smpool.tile([1, 1], F32, tag="m6")
            nc.vector.tensor_reduce(zsum[:], z[:], axis=mybir.AxisListType.X, op=ALU.add)
            nc.vector.reciprocal(zsum[:], zsum[:])
            nc.vector.tensor_scalar_mul(z[:], z[:], zsum[0:1, 0:1])
            cb_ps = psmall.tile([P, C], F32, tag="sm")
            nc.tensor.matmul(
                cb_ps[:, 0 : K + 1], lhsT=ones_row[:], rhs=z[:], start=True, stop=True
            )
            c_bc = cpool.tile([P, K + 1], F32)
            nc.vector.tensor_copy(c_bc[:], cb_ps[:, 0 : K + 1])

            # ---- phase B: K diffusion steps ----
            LQ16 = cfg.LQP // 16
            pt16 = cfg.piece_tok // 16
            for k in range(1, K + 1):
                src = table[k - 1]
                pieces = [[None] * cfg.n_pieces for _ in range(cfg.nq)]
                for i in range(cfg.n_pieces):
                    for q in range(cfg.nq):
                        gt = gpool.tile([P, cfg.piece_tok // P * C], F32, tag="g")
                        nc.gpsimd.dma_gather(
                            gt[:].rearrange("p (c f) -> p c f", f=C),
                            src[q * cfg.QS : (q + 1) * cfg.QS, :],
                            idxs[:, q * LQ16 + i * pt16 : q * LQ16 + (i + 1) * pt16],
                            num_idxs=cfg.piece_tok,
                            num_idxs_reg=cfg.piece_tok,
                            elem_size=C,
                            queue_num=q,
                        )
                        pieces[q][i] = gt

                s_stk = stpool.tile([P, FB], F32, tag="stage")
                ncc = cfg.nq * cfg.c_need
                for b in range(NB):
                    ps = pmain.tile([P, C], F32, tag="ps")
                    S = spool.tile([P, ncc * P], F32, tag="S")
                    nc.vector.tensor_tensor(
                        S[:].rearrange("p (c f) -> p c f", f=P),
                        dcols[:, b * ncc : (b + 1) * ncc].to_broadcast(
                            [P, ncc, P]
                        ),
                        iota_t[:].rearrange("p (c f) -> p c f", f=P),
                        op=ALU.is_equal,
                    )
                    for q in range(cfg.nq):
                        for c in range(cfg.c_need):
                            off = b * cfg.cap + c * P
                            gt = pieces[q][off // cfg.piece_tok]
                            gv = gt[:].rearrange("p (c f) -> p c f", f=C)
                            nc.tensor.matmul(
                                ps[:],
                                lhsT=S[:, (q * cfg.c_need + c) * P : (q * cfg.c_need + c + 1) * P],
                                rhs=gv[:, (off % cfg.piece_tok) // P, :],
                                start=(q == 0 and c == 0),
                                stop=(q == cfg.nq - 1 and c == cfg.c_need - 1),
                            )
                    # s_k = dis^2 * a_k
                    nc.scalar.activation(
                        s_stk[:, b * C : (b + 1) * C],
                        ps[:],
                        AF.Copy,
                        scale=dis2[:, b : b + 1],
                    )
                smk, stlk = dram_pf(s_loc[k])
                nc.sync.dma_start(
                    out=smk,
                    in_=s_stk[:, 0 : (NB - 1) * C].rearrange("p (b f) -> p b f", f=C),
                )
                nc.sync.dma_start(
                    out=stlk, in_=s_stk[0 : cfg.tail, (NB - 1) * C : FB]
                )
                if k < K:
                    nc.gpsimd.collective_compute(
                        "AllGather",
                        ALU.bypass,
                        replica_groups=groups,
                        ins=[s_loc[k][:]],
                        outs=[table[k][:]],
                    )

            # ---- phase C: combine ----
            acc = stpool.tile([P, FB], F32, tag="stage")
            for k in range(1, K + 1):
                sl = stpool.tile([P, FB], F32, tag="stage")
                nc.gpsimd.memset(sl[:, (NB - 1) * C : FB], 0.0)
                smk, stlk = dram_pf(s_loc[k])
                nc.sync.dma_start(
                    out=sl[:, 0 : (NB - 1) * C].rearrange("p (b f) -> p b f", f=C),
                    in_=smk,
                )
                nc.sync.dma_start(
                    out=sl[0 : cfg.tail, (NB - 1) * C : FB], in_=stlk
                )
                if k == 1:
                    for b in range(NB):
                        nc.vector.tensor_scalar_mul(
                            acc[:, b * C : (b + 1) * C],
                            sl[:, b * C : (b + 1) * C],
                            c_bc[:, k : k + 1],
                        )
                else:
                    nc.vector.tensor_scalar_mul(sl[:], sl[:], c_bc[:, k : k + 1])
                    nc.vector.tensor_tensor(acc[:], acc[:], sl[:], op=ALU.add)
            for b in range(NB):
                nc.vector.tensor_scalar_mul(
                    acc[:, b * C : (b + 1) * C],
                    acc[:, b * C : (b + 1) * C],
                    rdis[:, b : b + 1],
                )
            xl = stpool.tile([P, FB], F32, tag="stage")
            nc.gpsimd.memset(xl[:, (NB - 1) * C : FB], 0.0)
            xm2, xtl2 = dram_pf(xs_t)
            nc.sync.dma_start(
                out=xl[:, 0 : (NB - 1) * C].rearrange("p (b f) -> p b f", f=C), in_=xm2
            )
            nc.sync.dma_start(out=xl[0 : cfg.tail, (NB - 1) * C : FB], in_=xtl2)
            nc.vector.tensor_scalar_mul(xl[:], xl[:], c_bc[:, 0:1])
            nc.vector.tensor_tensor(acc[:], acc[:], xl[:], op=ALU.add)

            om, otl = dram_pf(out_t)
            nc.sync.dma_start(
                out=om, in_=acc[:, 0 : (NB - 1) * C].rearrange("p (b f) -> p b f", f=C)
            )
            nc.sync.dma_start(out=otl, in_=acc[0 : cfg.tail, (NB - 1) * C : FB])

    nc.finalize()
    return nc


_CACHE = {}


def _get_program(cfg: Cfg):
    if cfg not in _CACHE:
        _CACHE[cfg] = _build_program(cfg)
    return _CACHE[cfg]


def _run(inputs, trace=False, cfg_kw=None):
    from concourse.bass_utils import run_bass_kernel_spmd

    cfg_kw = dict(cfg_kw or FULL)
    x = np.asarray(inputs["x"], dtype=np.float32)
    cfg, in_maps = _preprocess(x, inputs["edge_index"], cfg_kw)
    W1 = np.asarray(inputs["W1"], dtype=np.float32)
    b1 = np.asarray(inputs["b1"], dtype=np.float32)
    W2 = np.asarray(inputs["W2"], dtype=np.float32)
    b2 = np.asarray(inputs["b2"], dtype=np.float32)
    for im in in_maps:
        im["w1t"] = np.ascontiguousarray(W1.T)
        im["b1c"] = np.ascontiguousarray(b1[:, None])
        im["w2t"] = np.ascontiguousarray(W2.T)
        im["b2r"] = np.ascontiguousarray(b2[None, :])
    nc = _get_program(cfg)
    res = run_bass_kernel_spmd(
        nc, in_maps, core_ids=list(range(cfg.ncores)), trace=trace
    )
    out = np.concatenate([res.results[i]["out"] for i in range(cfg.ncores)], axis=0)
    return out, res.exec_time_ns


def kernel(**inputs) -> np.ndarray:
    out, _ = _run(inputs)
    return out


# ---------------------------------------------------------------------------
# toy-scale validation against a numpy port of the reference, via CoreSim
# ---------------------------------------------------------------------------


def _np_reference(x, edge_index, W1, b1, W2, b2, K=5):
    N, C = x.shape
    E = edge_index.shape[1]
    row, col = edge_index[0].astype(np.int64), edge_index[1].astype(np.int64)
    deg = np.bincount(col, minlength=N).astype(np.float32)
    with np.errstate(divide="ignore"):
        dis = np.minimum(deg ** -0.5, 1e6).astype(np.float32)
    norm = dis[row] * dis[col]
    xm = x.mean(axis=0)
    stats = np.array([x.mean(), x.std(ddof=1), N, E], dtype=np.float32)
    cin = np.concatenate([xm, stats])
    h = np.maximum(W1 @ cin + b1, 0.0)
    zz = W2 @ h + b2
    zz = np.exp(zz - zz.max())
    coeffs = zz / zz.sum()
    result = coeffs[0] * x
    tx = x.copy()
    for k in range(1, K + 1):
        nt = np.zeros_like(tx)
        np.add.at(nt, row, norm[:, None] * tx[col])
        tx = nt
        result = result + coeffs[k] * tx
    return result


def _selftest_sim():
    from concourse.bass_interp import MultiCoreSim

    rng = np.random.default_rng(0)
    kw = dict(N=2400, E=9600, C=64, H=32, K=5, ncores=8, nq=4, piece_tok=384)
    x = rng.standard_normal((kw["N"], kw["C"])).astype(np.float32)
    ei = rng.integers(0, kw["N"], size=(2, kw["E"])).astype(np.int32)
    W1 = rng.uniform(-1, 1, (kw["H"], kw["C"] + 4)).astype(np.float32) / 8
    b1 = rng.uniform(-1, 1, (kw["H"],)).astype(np.float32) / 8
    W2 = rng.uniform(-1, 1, (kw["K"] + 1, kw["H"])).astype(np.float32) / 5
    b2 = rng.uniform(-1, 1, (kw["K"] + 1,)).astype(np.float32) / 5

    cfg, in_maps = _preprocess(x, ei, kw)
    print("toy cfg:", cfg)
    for im in in_maps:
        im["w1t"] = np.ascontiguousarray(W1.T)
        im["b1c"] = np.ascontiguousarray(b1[:, None])
        im["w2t"] = np.ascontiguousarray(W2.T)
        im["b2r"] = np.ascontiguousarray(b2[None, :])
    nc = _build_program(cfg)
    sim = MultiCoreSim(nc, cfg.ncores)
    for i in range(cfg.ncores):
        for name, arr in in_maps[i].items():
            sim.cores[i].tensor(name)[:] = arr
    sim.simulate()
    out = np.concatenate(
        [sim.cores[i].tensor("out") for i in range(cfg.ncores)], axis=0
    )
    exp = _np_reference(x, ei, W1, b1, W2, b2, K=kw["K"])
    err = np.abs(out - exp).max() / (np.abs(exp).max() + 1e-30)
    rel = np.linalg.norm(out - exp) / (np.linalg.norm(exp) + 1e-30)
    print(f"sim selftest: max-abs-rel {err:.3e}  fro-rel {rel:.3e}")
    assert rel < 1e-4, (rel, err)
    print("SIM SELFTEST PASSED")


if __name__ == "__main__":
    _selftest_sim()

